# revision 15
# baseline (speedup 1.0000x reference)
"""Contrastive loss (SimCLR-style NT-Xent, faithful variant) on 8 Trainium2 cores.

Problem: x1, x2 [4096, 256] f32.  z = normalize(concat(x1, x2)) [8192, 256];
sim = z @ z.T; pos = diag(sim, +4096) used for both halves;
den_g = sum_j exp(mask_offdiag * sim_gj / tau)  (diag contributes exp(0)=1);
loss = mean(log(den) - pos_pairs/tau).

Sharding (symmetric-pair cover): exp(sim/tau) is symmetric, so each unordered
block pair {a, b} of the 8x8 grid of 1024x1024 tiles is computed ONCE.  Core c
owns row-block c and computes tiles {c, c+k mod 8} for k = 0..4 (the k=4 tile
is valid only for c < 4; cores 4-7 compute it redundantly and the host drops
it).  The host receives, per core: per-row partial sums (row side), the
column-compressed exp tiles (column side, folded over the 8 row sub-tiles on
the vector engines), the diagonal correction exp(selfsim/tau), and the
positive-pair partial sum.  The host plays the role of the all-reduce: it
scatter-adds partials into den[8192], applies +1 - selfexp, and finishes
loss = (sum log den - sum pos/tau) / 2N.  Device work per core drops to 5/8
of the full row-block (the scalar-engine exp is the kernel's critical
resource).

Inputs are host-rotated so the per-core tile set lives at the SAME local
column offsets on every core (SPMD program uniformity): core c receives
xT rolled left by c*1024 columns, truncated to 5120 columns, in bf16.

Other device-side structure:
- ln+exp are forced into the one activation-table set holding both (a single
  ACT_TABLE_LOAD for the whole kernel).
- the GEMM lhsT is the RAW own-row block; the row-side 1/||x_i|| scale rides
  the exp activation's per-partition scale operand (10 * rsqrt in an [p, m]
  layout).  Only the moving side (zt) is materialized normalized.
- column rsqrt for superblock 0 is computed scalar-direct (ln of the PSUM
  sumsq row, ones-matmul broadcast of the ln, then a fused exp(-0.5 x)); for
  superblocks 1-2 the sumsq row is folded [1, C] -> [128, C/128] through a
  contiguous DRAM bounce so the scalar engine only runs two ~300ns
  activations per block.
"""

import numpy as np

import concourse.bass as bass
import concourse.tile as tile
from concourse import bacc, mybir

F32 = mybir.dt.float32
F32R = mybir.dt.float32r
BF16 = mybir.dt.bfloat16
AF = mybir.ActivationFunctionType
ALU = mybir.AluOpType
AX = mybir.AxisListType
PSUM = bass.MemorySpace.PSUM

N = 4096
TWO_N = 2 * N
RPC = 1024                # rows per core
COLS = 5 * RPC            # local columns touched per core
TAU_INV = 10.0
LN10 = float(np.log(10.0))
M_TILES = 8               # 128-row sub-tiles per core
NBW = [2048, 2048, 1024]  # local column superblocks


def _patch_act_tables():
    """Force ln+exp into natural_log_exp_and_others (one table load)."""
    import concourse.bacc as _bacc
    import concourse.hw_specs as _hw

    orig = _hw.get_activation_tables

    def patched(arch):
        tables = dict(orig(arch))
        ln = mybir.ActivationFunctionType.Ln
        exp = mybir.ActivationFunctionType.Exp
        out = {}
        for name, funcs in tables.items():
            if name != "natural_log_exp_and_others" and (
                ln in funcs or exp in funcs
            ):
                funcs = funcs - {ln, exp}
            out[name] = funcs
        return out

    _bacc.get_activation_tables = patched


def build_nc(nc=None):
    _patch_act_tables()
    if nc is None:
        nc = bacc.Bacc("TRN2", target_bir_lowering=False, debug=False)

    xt = [
        nc.declare_dram_parameter(f"xt{k}", [128, COLS], BF16, isOutput=False)
        for k in range(2)
    ]
    rden_d = nc.declare_dram_parameter("rden", [128, M_TILES * 3], F32, isOutput=True)
    cden_d = nc.declare_dram_parameter("cden", [128, COLS], F32, isOutput=True)
    seout_d = nc.declare_dram_parameter("seout", [1, RPC], F32, isOutput=True)
    pos_d = nc.declare_dram_parameter("poso", [1, 1], F32, isOutput=True)

    with tile.TileContext(nc) as tc:
        with (
            tc.tile_pool(name="const", bufs=1) as cpool,
            tc.tile_pool(name="xt", bufs=1) as xt_pool,
            tc.tile_pool(name="zt", bufs=1) as zt_pool,
            tc.tile_pool(name="rows", bufs=1) as row_pool,
            tc.tile_pool(name="xsq", bufs=4) as xsq_pool,
            tc.tile_pool(name="bcs", bufs=1) as bcs_pool,
            tc.tile_pool(name="esb", bufs=3) as esb_pool,
            tc.tile_pool(name="cac", bufs=1) as cac_pool,
            tc.tile_pool(name="fin", bufs=1) as fin_pool,
            tc.tile_pool(name="dram", bufs=1, space="DRAM") as dram_pool,
        ):
            ones_col32 = cpool.tile([128, 1], F32, name="ones_col32", tag="ones_col32")
            nc.vector.memset(ones_col32[:], 1.0)
            ones_col = cpool.tile([128, 1], F32R, name="ones_col", tag="ones_col")
            nc.vector.tensor_copy(ones_col[:], ones_col32[:])
            ones_col_bf = cpool.tile([128, 1], BF16, name="ones_col_bf", tag="ones_col_bf")
            nc.vector.tensor_copy(ones_col_bf[:], ones_col32[:])
            ones_row_bf = cpool.tile([1, 128], BF16, name="ones_row_bf", tag="ones_row_bf")
            nc.vector.memset(ones_row_bf[:], 1.0)
            ones_row_32 = cpool.tile([1, 128], F32, name="ones_row_32", tag="ones_row_32")
            nc.vector.memset(ones_row_32[:], 1.0)
            ones_row_r = cpool.tile([1, 128], F32R, name="ones_row_r", tag="ones_row_r")
            nc.vector.tensor_copy(ones_row_r[:], ones_row_32[:])
            ln10_col = cpool.tile([128, 1], F32, name="ln10_col", tag="ln10_col")
            nc.vector.memset(ln10_col[:], LN10)

            # per-superblock raw and normalized tiles
            xt_sb = [
                [
                    xt_pool.tile([128, NBW[b]], BF16, name=f"xt{k}_{b}", tag=f"xt{k}_{b}")
                    for b in range(3)
                ]
                for k in range(2)
            ]
            zt_sb = [
                [
                    zt_pool.tile([128, NBW[b]], BF16, name=f"zt{k}_{b}", tag=f"zt{k}_{b}")
                    for b in range(3)
                ]
                for k in range(2)
            ]

            ln0_row = row_pool.tile([1, 2048], F32R, name="ln0_row", tag="ln0_row")
            ss1_row = row_pool.tile([1, 2048], F32, name="ss1_row", tag="ss1_row")
            ss2_row = row_pool.tile([1, 1024], F32, name="ss2_row", tag="ss2_row")
            rsq1_row = row_pool.tile([1, 2048], BF16, name="rsq1_row", tag="rsq1_row")
            rsq2_row = row_pool.tile([1, 1024], BF16, name="rsq2_row", tag="rsq2_row")
            ra10_t = row_pool.tile([128, M_TILES], F32, name="ra10_t", tag="ra10_t")

            bc_sb = [
                bcs_pool.tile([128, NBW[b]], BF16, name=f"bc_{b}", tag=f"bc_{b}")
                for b in range(3)
            ]
            cacc = [
                cac_pool.tile([128, NBW[b]], F32, name=f"cacc{b}", tag=f"cacc{b}")
                for b in range(3)
            ]
            cacc2 = [
                cac_pool.tile([128, NBW[b]], F32, name=f"cacc2{b}", tag=f"cacc2{b}")
                for b in range(3)
            ]

            den_acc = fin_pool.tile(
                [128, M_TILES * 3], F32, name="den_acc", tag="den_acc"
            )
            selfexp_row = fin_pool.tile(
                [1, RPC], F32, name="selfexp_row", tag="selfexp_row"
            )
            possum = fin_pool.tile([1, 1], F32, name="possum", tag="possum")

            ln_d = dram_pool.tile([1, RPC], F32R, name="ln_d", tag="ln_d")
            ss1_d = dram_pool.tile([1, 2048], F32, name="ss1_d", tag="ss1_d")
            ss2_d = dram_pool.tile([1, 1024], F32, name="ss2_d", tag="ss2_d")
            rsq1_d = dram_pool.tile([128, 16], BF16, name="rsq1_d", tag="rsq1_d")
            rsq2_d = dram_pool.tile([128, 8], BF16, name="rsq2_d", tag="rsq2_d")

            # ---- input DMAs: sb0 first (it gates everything) ----
            off = 0
            for b in range(3):
                cs = slice(off, off + NBW[b])
                for k in range(2):
                    eng = nc.sync if k == 0 else nc.scalar
                    eng.dma_start(xt_sb[k][b][:], xt[k][:, cs])
                off += NBW[b]

            with (
                tc.tile_pool(name="ssp", bufs=1, space=PSUM) as ss_pool,
                tc.tile_pool(name="bcp", bufs=1, space=PSUM) as bc_pool,
            ):

                def sumsq(b, eng0, eng1):
                    """-> [1, NBW[b]] PSUM sumsq of superblock b (both k halves)."""
                    w = NBW[b]
                    xsq = [
                        xsq_pool.tile([128, 2048], BF16, name="xsq", tag="xsq")[:, 0:w]
                        for k in range(2)
                    ]
                    eng0.tensor_mul(xsq[0][:], xt_sb[0][b][:], xt_sb[0][b][:])
                    eng1.tensor_mul(xsq[1][:], xt_sb[1][b][:], xt_sb[1][b][:])
                    ss = ss_pool.tile([1, 2048], F32, name="ss", tag="ss")
                    for j in range(w // 512):
                        js = slice(j * 512, (j + 1) * 512)
                        for k in range(2):
                            nc.tensor.matmul(
                                ss[0:1, js],
                                ones_col_bf[:],
                                xsq[k][:, js],
                                start=(k == 0),
                                stop=(k == 1),
                            )
                    return ss

                def fold_rsq(ss_row_t, rsq_row_t, width, ss_d, rsq_d, tagc):
                    """rsq_row = rsqrt(ss_row) via a [128, width/128] fold."""
                    m = width // 128
                    nc.sync.dma_start(ss_d[:], ss_row_t[:])
                    ss_t = row_pool.tile(
                        [128, m], F32, name=f"ss_t{tagc}", tag=f"ss_t{tagc}"
                    )
                    nc.sync.dma_start(
                        ss_t[:], ss_d[0:1, :].rearrange("o (p m) -> (o p) m", p=128)
                    )
                    ln_t = row_pool.tile(
                        [128, m], F32, name=f"ln_t{tagc}", tag=f"ln_t{tagc}"
                    )
                    nc.scalar.activation(ln_t[:], ss_t[:], AF.Ln)
                    rsq_t = row_pool.tile(
                        [128, m], BF16, name=f"rsq_t{tagc}", tag=f"rsq_t{tagc}"
                    )
                    nc.scalar.activation(rsq_t[:], ln_t[:], AF.Exp, scale=-0.5)
                    nc.sync.dma_start(rsq_d[:], rsq_t[:])
                    nc.sync.dma_start(
                        rsq_row_t[:],
                        rsq_d[:, :].rearrange("(o p) m -> o (p m)", p=128),
                    )

                # -- superblock 0: scalar-direct normalize (shortest chain) --
                ss0 = sumsq(0, nc.vector, nc.gpsimd)
                nc.scalar.activation(ln0_row[:], ss0[:], AF.Ln)
                bc_ps = bc_pool.tile([128, 2048], F32, name="bcp", tag="bcp")
                for j in range(4):
                    js = slice(j * 512, (j + 1) * 512)
                    nc.tensor.matmul(
                        bc_ps[:, js],
                        ones_row_r[:],
                        ln0_row[0:1, js],
                        start=True,
                        stop=True,
                    )
                # bc = exp(-0.5 * broadcast(ln)) = rsqrt(ss), straight to SBUF
                nc.scalar.activation(bc_sb[0][:], bc_ps[:], AF.Exp, scale=-0.5)
                for k in range(2):
                    eng = nc.vector if k == 0 else nc.gpsimd
                    eng.tensor_mul(zt_sb[k][0][:], xt_sb[k][0][:], bc_sb[0][:])

                # row-side scale: ra10[p, m] = 10 * rsqrt(ss_row0[m*128+p])
                nc.sync.dma_start(ln_d[:], ln0_row[0:1, 0:RPC])
                ln_mp = row_pool.tile([128, M_TILES], F32R, name="ln_mp", tag="ln_mp")
                nc.sync.dma_start(
                    ln_mp[:], ln_d[0:1, :].rearrange("o (m p) -> (o p) m", p=128)
                )
                nc.scalar.activation(
                    ra10_t[:], ln_mp[:], AF.Exp, scale=-0.5, bias=ln10_col[:]
                )

                # prods for pos / selfsim (reduced mid-main through a slot)
                prod_a = [
                    xsq_pool.tile(
                        [128, RPC], F32R, name=f"prod_a{k}", tag=f"prod_a{k}", bufs=1
                    )
                    for k in range(2)
                ]
                prod_s = [
                    xsq_pool.tile(
                        [128, RPC], F32R, name=f"prod_s{k}", tag=f"prod_s{k}", bufs=1
                    )
                    for k in range(2)
                ]

            # ---- main loop: 5 local col blocks x 8 row tiles ----
            # sumsq/broadcast for superblock nb+1 and the pos/selfsim
            # reductions ride spare sim-pool rotation slots so the prep PSUM
            # pools can close before the loop (PSUM is fully consumed by the
            # two [128, 2048] sim tiles).
            with tc.tile_pool(name="simp", bufs=2, space=PSUM) as sim_pool:
                ss_rows = {1: ss1_row, 2: ss2_row}
                rsq_rows = {1: rsq1_row, 2: rsq2_row}
                ss_ds = {1: ss1_d, 2: ss2_d}
                rsq_ds = {1: rsq1_d, 2: rsq2_d}

                def slot_ss(b):
                    """sumsq of superblock b through a sim slot + fold bounce."""
                    w = NBW[b]
                    xsq = [
                        xsq_pool.tile([128, 2048], BF16, name="xsq", tag="xsq")[:, 0:w]
                        for k in range(2)
                    ]
                    nc.vector.tensor_mul(xsq[0][:], xt_sb[0][b][:], xt_sb[0][b][:])
                    nc.gpsimd.tensor_mul(xsq[1][:], xt_sb[1][b][:], xt_sb[1][b][:])
                    slot = sim_pool.tile([128, 2048], F32, name="sim", tag="sim")
                    for j in range(w // 512):
                        js = slice(j * 512, (j + 1) * 512)
                        for k in range(2):
                            nc.tensor.matmul(
                                slot[0:1, js],
                                ones_col_bf[:],
                                xsq[k][:, js],
                                start=(k == 0),
                                stop=(k == 1),
                            )
                    nc.vector.tensor_copy(ss_rows[b][:], slot[0:1, 0:w])
                    fold_rsq(
                        ss_rows[b], rsq_rows[b], w, ss_ds[b], rsq_ds[b], f"f{b}"
                    )

                def slot_bc(b):
                    """broadcast rsq row of superblock b + normalize muls."""
                    w = NBW[b]
                    slot = sim_pool.tile([128, 2048], F32, name="sim", tag="sim")
                    for j in range(w // 512):
                        js = slice(j * 512, (j + 1) * 512)
                        nc.tensor.matmul(
                            slot[:, js],
                            ones_row_bf[:],
                            rsq_rows[b][0:1, js],
                            start=True,
                            stop=True,
                        )
                    nc.vector.tensor_copy(bc_sb[b][:], slot[:, 0:w])
                    for k in range(2):
                        eng = nc.vector if k == 0 else nc.gpsimd
                        eng.tensor_mul(zt_sb[k][b][:], xt_sb[k][b][:], bc_sb[b][:])

                def slot_posself():
                    """pos / selfsim partition reductions through one slot."""
                    for k in range(2):
                        nc.vector.tensor_mul(
                            prod_a[k][:], zt_sb[k][0][:, 0:RPC], zt_sb[k][2][:, 0:RPC]
                        )
                        nc.gpsimd.tensor_mul(
                            prod_s[k][:], xt_sb[k][0][:, 0:RPC], zt_sb[k][0][:, 0:RPC]
                        )
                    slot = sim_pool.tile([128, 2048], F32, name="sim", tag="sim")
                    for j in range(2):
                        js = slice(j * 512, (j + 1) * 512)
                        js2 = slice(1024 + j * 512, 1024 + (j + 1) * 512)
                        for k in range(2):
                            nc.tensor.matmul(
                                slot[0:1, js],
                                ones_col[:],
                                prod_a[k][:, js],
                                start=(k == 0),
                                stop=(k == 1),
                            )
                        for k in range(2):
                            nc.tensor.matmul(
                                slot[0:1, js2],
                                ones_col[:],
                                prod_s[k][:, js],
                                start=(k == 0),
                                stop=(k == 1),
                            )
                    nc.vector.tensor_reduce(
                        possum[:], slot[0:1, 0:1024], axis=AX.X, op=ALU.add
                    )
                    # selfexp_row = exp(selfsim * 10 * rsq_i); rsq row for the
                    # own block is partition 0 of the sb0 broadcast tile.
                    t1 = row_pool.tile([1, RPC], F32, name="t1", tag="t1")
                    nc.vector.tensor_mul(
                        t1[:], slot[0:1, 1024:2048], bc_sb[0][0:1, 0:RPC]
                    )
                    nc.scalar.activation(
                        selfexp_row[:], t1[:], AF.Exp, scale=TAU_INV
                    )

                for nb in range(3):
                    w = NBW[nb]
                    if nb < 2:
                        slot_ss(nb + 1)      # bounce runs during this nb's GEMM
                    else:
                        slot_posself()
                    cv = [None, None]
                    for m in range(M_TILES):
                        ms = slice(m * 128, (m + 1) * 128)
                        st = sim_pool.tile([128, 2048], F32, name="sim", tag="sim")
                        for k in range(2):
                            for j4 in range(w // 512):
                                js = slice(j4 * 512, (j4 + 1) * 512)
                                nc.tensor.matmul(
                                    st[:, js],
                                    xt_sb[k][0][:, ms],
                                    zt_sb[k][nb][:, js],
                                    start=(k == 0),
                                    stop=(k == 1),
                                )
                        e_sb = esb_pool.tile([128, 2048], BF16, name="esb", tag="esb")
                        idx = m * 3 + nb
                        nc.scalar.activation(
                            e_sb[:, 0:w],
                            st[:, 0:w],
                            AF.Exp,
                            scale=ra10_t[:, m : m + 1],
                            accum_out=den_acc[:, idx : idx + 1],
                        )
                        # two independent fold chains (vector / gpsimd)
                        half = m % 2
                        eng = nc.vector if half == 0 else nc.gpsimd
                        ct = cacc[nb] if half == 0 else cacc2[nb]
                        if cv[half] is None:
                            eng.tensor_copy(ct[:, 0:w], e_sb[:, 0:w])
                            cv[half] = True
                        else:
                            eng.tensor_tensor(
                                ct[:, 0:w], ct[:, 0:w], e_sb[:, 0:w], op=ALU.add
                            )
                    nc.vector.tensor_tensor(
                        cacc[nb][:, 0:w], cacc[nb][:, 0:w], cacc2[nb][:, 0:w],
                        op=ALU.add,
                    )
                    if nb < 2:
                        slot_bc(nb + 1)
                    off = sum(NBW[:nb])
                    nc.sync.dma_start(cden_d[:, off : off + w], cacc[nb][:])

            # ---- outputs ----
            nc.sync.dma_start(rden_d[:], den_acc[:])
            nc.sync.dma_start(seout_d[:], selfexp_row[:])
            nc.sync.dma_start(pos_d[:], possum[:])

    nc.compile()
    return nc


_NC = None


def _get_nc():
    global _NC
    if _NC is None:
        _NC = build_nc()
    return _NC


def make_in_maps(x1, x2):
    import ml_dtypes

    x1 = np.asarray(x1, dtype=np.float32)
    x2 = np.asarray(x2, dtype=np.float32)
    x = np.concatenate([x1, x2], axis=0)              # [8192, 256]
    xT = np.ascontiguousarray(x.T).astype(ml_dtypes.bfloat16)  # [256, 8192]
    in_maps = []
    for c in range(8):
        rot = np.roll(xT, -c * RPC, axis=1)[:, 0:COLS]
        in_maps.append(
            {
                "xt0": np.ascontiguousarray(rot[:128]),
                "xt1": np.ascontiguousarray(rot[128:]),
            }
        )
    return in_maps


def _reduce_host(results):
    """Stand-in for the all-reduce: scatter-add the per-core partials and
    finish the scalar loss."""
    den = np.zeros(TWO_N, dtype=np.float64)
    pos_tot = 0.0
    for c in range(8):
        r = results[c]
        nmax = 3 if c < 4 else 2          # nb=2 (block pair {c, c+4}) owner: c<4
        rden = np.asarray(r["rden"], dtype=np.float64).reshape(128, M_TILES, 3)
        contrib = rden[:, :, 0:nmax].sum(axis=2)        # [p, m]
        den[c * RPC : (c + 1) * RPC] += contrib.T.reshape(RPC)  # row = m*128+p
        colsum = np.asarray(r["cden"], dtype=np.float64).sum(axis=0)  # [5120]
        bmax = 5 if c < 4 else 4
        for b in range(1, bmax):          # b=0 is the diagonal tile: row side only
            g0 = ((c + b) % 8) * RPC
            den[g0 : g0 + RPC] += colsum[b * RPC : (b + 1) * RPC]
        seout = np.asarray(r["seout"], dtype=np.float64).reshape(RPC)
        den[c * RPC : (c + 1) * RPC] += 1.0 - seout
        pos_tot += float(np.asarray(r["poso"])[0, 0])
    loss = (np.log(den).sum() - TAU_INV * pos_tot) / TWO_N
    return np.asarray(np.float32(loss))


def _run(x1, x2, trace=False, tmpdir=None):
    from concourse.bass_utils import run_bass_kernel_spmd

    nc = _get_nc()
    in_maps = make_in_maps(x1, x2)
    res = run_bass_kernel_spmd(
        nc, in_maps, list(range(8)), trace=trace, tmpdir=tmpdir
    )
    loss = _reduce_host(res.results)
    return loss, res


def kernel(x1, x2):
    loss, _ = _run(x1, x2)
    return loss


# revision 16
# speedup vs baseline: 1.2173x; 1.2173x over previous
"""Contrastive loss (SimCLR-style NT-Xent, faithful variant) on 8 Trainium2 cores.

Problem: x1, x2 [4096, 256] f32.  z = normalize(concat(x1, x2)) [8192, 256];
sim = z @ z.T; pos = diag(sim, +4096) used for both halves;
den_g = sum_j exp(mask_offdiag * sim_gj / tau)  (diag contributes exp(0)=1);
loss = mean(log(den) - pos_pairs/tau).

Sharding (symmetric-pair cover): exp(sim/tau) is symmetric, so each unordered
block pair {a, b} of the 8x8 grid of 1024x1024 tiles is computed ONCE.  Core c
owns row-block c and computes tiles {c, c+k mod 8} for k = 0..4 (the k=4 tile
is valid only for c < 4; cores 4-7 compute it redundantly and the host drops
it).  The host receives, per core: per-row partial sums (row side), the
column-compressed exp tiles (column side, folded over the 8 row sub-tiles on
the vector engines), the diagonal correction exp(selfsim/tau), and the
positive-pair partial sum.  The host plays the role of the all-reduce: it
scatter-adds partials into den[8192], applies +1 - selfexp, and finishes
loss = (sum log den - sum pos/tau) / 2N.  Device work per core drops to 5/8
of the full row-block (the scalar-engine exp is the kernel's critical
resource).

Inputs are host-rotated so the per-core tile set lives at the SAME local
column offsets on every core (SPMD program uniformity): core c receives
xT rolled left by c*1024 columns, truncated to 5120 columns, in bf16.

Other device-side structure:
- ln+exp are forced into the one activation-table set holding both (a single
  ACT_TABLE_LOAD for the whole kernel).
- the GEMM lhsT is the RAW own-row block; the row-side 1/||x_i|| scale rides
  the exp activation's per-partition scale operand (10 * rsqrt in an [p, m]
  layout).  Only the moving side (zt) is materialized normalized.
- column rsqrt for superblock 0 is computed scalar-direct (ln of the PSUM
  sumsq row, ones-matmul broadcast of the ln, then a fused exp(-0.5 x)); for
  superblocks 1-2 the sumsq row is folded [1, C] -> [128, C/128] through a
  contiguous DRAM bounce so the scalar engine only runs two ~300ns
  activations per block.
"""

import numpy as np

import concourse.bass as bass
import concourse.tile as tile
from concourse import bacc, mybir

F32 = mybir.dt.float32
F32R = mybir.dt.float32r
BF16 = mybir.dt.bfloat16
AF = mybir.ActivationFunctionType
ALU = mybir.AluOpType
AX = mybir.AxisListType
PSUM = bass.MemorySpace.PSUM

N = 4096
TWO_N = 2 * N
RPC = 1024                # rows per core
COLS = 5 * RPC            # local columns touched per core
TAU_INV = 10.0
LN10 = float(np.log(10.0))
M_TILES = 8               # 128-row sub-tiles per core
NBW = [2048, 2048, 1024]  # local column superblocks


def _patch_act_tables():
    """Force ln+exp into natural_log_exp_and_others (one table load)."""
    import concourse.bacc as _bacc
    import concourse.hw_specs as _hw

    orig = _hw.get_activation_tables

    def patched(arch):
        tables = dict(orig(arch))
        ln = mybir.ActivationFunctionType.Ln
        exp = mybir.ActivationFunctionType.Exp
        out = {}
        for name, funcs in tables.items():
            if name != "natural_log_exp_and_others" and (
                ln in funcs or exp in funcs
            ):
                funcs = funcs - {ln, exp}
            out[name] = funcs
        return out

    _bacc.get_activation_tables = patched


def build_nc(nc=None):
    _patch_act_tables()
    if nc is None:
        nc = bacc.Bacc("TRN2", target_bir_lowering=False, debug=False)

    xt = [
        nc.declare_dram_parameter(f"xt{k}", [128, COLS], BF16, isOutput=False)
        for k in range(2)
    ]
    rden_d = nc.declare_dram_parameter("rden", [128, M_TILES * 3], F32, isOutput=True)
    cden_d = nc.declare_dram_parameter("cden", [128, COLS], F32, isOutput=True)
    seout_d = nc.declare_dram_parameter("seout", [1, RPC], F32, isOutput=True)
    pos_d = nc.declare_dram_parameter("poso", [1, 1], F32, isOutput=True)
    ra_d = nc.declare_dram_parameter("raout", [128, 8], F32, isOutput=True)

    with tile.TileContext(nc) as tc:
        with (
            tc.tile_pool(name="const", bufs=1) as cpool,
            tc.tile_pool(name="xt", bufs=1) as xt_pool,
            tc.tile_pool(name="zt", bufs=1) as zt_pool,
            tc.tile_pool(name="rows", bufs=1) as row_pool,
            tc.tile_pool(name="xsq", bufs=4) as xsq_pool,
            tc.tile_pool(name="bcs", bufs=1) as bcs_pool,
            tc.tile_pool(name="esb", bufs=3) as esb_pool,
            tc.tile_pool(name="cac", bufs=1) as cac_pool,
            tc.tile_pool(name="fin", bufs=1) as fin_pool,
            tc.tile_pool(name="dram", bufs=1, space="DRAM") as dram_pool,
        ):
            ones_col32 = cpool.tile([128, 1], F32, name="ones_col32", tag="ones_col32")
            nc.vector.memset(ones_col32[:], 1.0)
            ones_col = cpool.tile([128, 1], F32R, name="ones_col", tag="ones_col")
            nc.vector.tensor_copy(ones_col[:], ones_col32[:])
            ones_col_bf = cpool.tile([128, 1], BF16, name="ones_col_bf", tag="ones_col_bf")
            nc.vector.tensor_copy(ones_col_bf[:], ones_col32[:])
            ones_row_bf = cpool.tile([1, 128], BF16, name="ones_row_bf", tag="ones_row_bf")
            nc.vector.memset(ones_row_bf[:], 1.0)
            ones_row_32 = cpool.tile([1, 128], F32, name="ones_row_32", tag="ones_row_32")
            nc.vector.memset(ones_row_32[:], 1.0)
            ones_row_r = cpool.tile([1, 128], F32R, name="ones_row_r", tag="ones_row_r")
            nc.vector.tensor_copy(ones_row_r[:], ones_row_32[:])
            ln10_col = cpool.tile([128, 1], F32, name="ln10_col", tag="ln10_col")
            nc.vector.memset(ln10_col[:], LN10)

            # per-superblock raw and normalized tiles
            xt_sb = [
                [
                    xt_pool.tile([128, NBW[b]], BF16, name=f"xt{k}_{b}", tag=f"xt{k}_{b}")
                    for b in range(3)
                ]
                for k in range(2)
            ]
            zt_sb = [
                [
                    zt_pool.tile([128, NBW[b]], BF16, name=f"zt{k}_{b}", tag=f"zt{k}_{b}")
                    for b in range(3)
                ]
                for k in range(2)
            ]

            ln0_row = row_pool.tile([1, 2048], F32R, name="ln0_row", tag="ln0_row")
            ss1_row = row_pool.tile([1, 2048], F32, name="ss1_row", tag="ss1_row")
            ss2_row = row_pool.tile([1, 1024], F32, name="ss2_row", tag="ss2_row")
            rsq1_row = row_pool.tile([1, 2048], BF16, name="rsq1_row", tag="rsq1_row")
            rsq2_row = row_pool.tile([1, 1024], BF16, name="rsq2_row", tag="rsq2_row")
            ra10_t = row_pool.tile([128, M_TILES], F32, name="ra10_t", tag="ra10_t")

            bc_sb = [
                bcs_pool.tile([128, NBW[b]], BF16, name=f"bc_{b}", tag=f"bc_{b}")
                for b in range(3)
            ]
            cacc = [
                cac_pool.tile([128, NBW[b]], F32, name=f"cacc{b}", tag=f"cacc{b}")
                for b in range(3)
            ]

            den_acc = fin_pool.tile(
                [128, M_TILES * 3], F32, name="den_acc", tag="den_acc"
            )
            selfexp_row = fin_pool.tile(
                [1, RPC], F32, name="selfexp_row", tag="selfexp_row"
            )
            possum = fin_pool.tile([1, 1], F32, name="possum", tag="possum")

            ln_d = dram_pool.tile([1, RPC], F32R, name="ln_d", tag="ln_d")
            ss1_d = dram_pool.tile([1, 2048], F32, name="ss1_d", tag="ss1_d")
            ss2_d = dram_pool.tile([1, 1024], F32, name="ss2_d", tag="ss2_d")
            rsq1_d = dram_pool.tile([128, 16], BF16, name="rsq1_d", tag="rsq1_d")
            rsq2_d = dram_pool.tile([128, 8], BF16, name="rsq2_d", tag="rsq2_d")

            # ---- input DMAs: sb0 first (it gates everything) ----
            off = 0
            for b in range(3):
                cs = slice(off, off + NBW[b])
                for k in range(2):
                    eng = nc.sync if k == 0 else nc.scalar
                    eng.dma_start(xt_sb[k][b][:], xt[k][:, cs])
                off += NBW[b]

            with (
                tc.tile_pool(name="ssp", bufs=1, space=PSUM) as ss_pool,
                tc.tile_pool(name="bcp", bufs=1, space=PSUM) as bc_pool,
            ):

                def sumsq(b, eng0, eng1):
                    """-> [1, NBW[b]] PSUM sumsq of superblock b (both k halves)."""
                    w = NBW[b]
                    xsq = [
                        xsq_pool.tile([128, 2048], BF16, name="xsq", tag="xsq")[:, 0:w]
                        for k in range(2)
                    ]
                    eng0.tensor_mul(xsq[0][:], xt_sb[0][b][:], xt_sb[0][b][:])
                    eng1.tensor_mul(xsq[1][:], xt_sb[1][b][:], xt_sb[1][b][:])
                    ss = ss_pool.tile([1, 2048], F32, name="ss", tag="ss")
                    for j in range(w // 512):
                        js = slice(j * 512, (j + 1) * 512)
                        for k in range(2):
                            nc.tensor.matmul(
                                ss[0:1, js],
                                ones_col_bf[:],
                                xsq[k][:, js],
                                start=(k == 0),
                                stop=(k == 1),
                            )
                    return ss

                def fold_rsq(ss_row_t, rsq_row_t, width, ss_d, rsq_d, tagc):
                    """rsq_row = rsqrt(ss_row) via a [128, width/128] fold."""
                    m = width // 128
                    nc.sync.dma_start(ss_d[:], ss_row_t[:])
                    ss_t = row_pool.tile(
                        [128, m], F32, name=f"ss_t{tagc}", tag=f"ss_t{tagc}"
                    )
                    nc.sync.dma_start(
                        ss_t[:], ss_d[0:1, :].rearrange("o (p m) -> (o p) m", p=128)
                    )
                    ln_t = row_pool.tile(
                        [128, m], F32, name=f"ln_t{tagc}", tag=f"ln_t{tagc}"
                    )
                    nc.scalar.activation(ln_t[:], ss_t[:], AF.Ln)
                    rsq_t = row_pool.tile(
                        [128, m], BF16, name=f"rsq_t{tagc}", tag=f"rsq_t{tagc}"
                    )
                    nc.scalar.activation(rsq_t[:], ln_t[:], AF.Exp, scale=-0.5)
                    nc.sync.dma_start(rsq_d[:], rsq_t[:])
                    nc.sync.dma_start(
                        rsq_row_t[:],
                        rsq_d[:, :].rearrange("(o p) m -> o (p m)", p=128),
                    )

                # -- superblock 0: scalar-direct normalize (shortest chain) --
                ss0 = sumsq(0, nc.vector, nc.gpsimd)
                nc.scalar.activation(ln0_row[:], ss0[:], AF.Ln)
                bc_ps = bc_pool.tile([128, 2048], F32, name="bcp", tag="bcp")
                for j in range(4):
                    js = slice(j * 512, (j + 1) * 512)
                    nc.tensor.matmul(
                        bc_ps[:, js],
                        ones_row_r[:],
                        ln0_row[0:1, js],
                        start=True,
                        stop=True,
                    )
                # bc = exp(-0.5 * broadcast(ln)) = rsqrt(ss), straight to SBUF
                nc.scalar.activation(bc_sb[0][:], bc_ps[:], AF.Exp, scale=-0.5)
                for k in range(2):
                    eng = nc.vector if k == 0 else nc.gpsimd
                    eng.tensor_mul(zt_sb[k][0][:], xt_sb[k][0][:], bc_sb[0][:])

                # row-side scale: ra10[p, m] = 10 * rsqrt(ss_row0[m*128+p])
                nc.sync.dma_start(ln_d[:], ln0_row[0:1, 0:RPC])
                ln_mp = row_pool.tile([128, M_TILES], F32R, name="ln_mp", tag="ln_mp")
                nc.sync.dma_start(
                    ln_mp[:], ln_d[0:1, :].rearrange("o (m p) -> (o p) m", p=128)
                )
                nc.scalar.activation(
                    ra10_t[:], ln_mp[:], AF.Exp, scale=-0.5, bias=ln10_col[:]
                )

                # prods for pos / selfsim (reduced mid-main through a slot)
                prod_a = [
                    xsq_pool.tile(
                        [128, RPC], F32R, name=f"prod_a{k}", tag=f"prod_a{k}", bufs=1
                    )
                    for k in range(2)
                ]
                prod_s = [
                    xsq_pool.tile(
                        [128, RPC], F32R, name=f"prod_s{k}", tag=f"prod_s{k}", bufs=1
                    )
                    for k in range(2)
                ]

            # ---- main loop: 5 local col blocks x 8 row tiles ----
            # sumsq/broadcast for superblock nb+1 and the pos/selfsim
            # reductions ride spare sim-pool rotation slots so the prep PSUM
            # pools can close before the loop (PSUM is fully consumed by the
            # two [128, 2048] sim tiles).
            with tc.tile_pool(name="simp", bufs=2, space=PSUM) as sim_pool:
                ss_rows = {1: ss1_row, 2: ss2_row}
                rsq_rows = {1: rsq1_row, 2: rsq2_row}
                ss_ds = {1: ss1_d, 2: ss2_d}
                rsq_ds = {1: rsq1_d, 2: rsq2_d}

                def slot_ss(b):
                    """sumsq of superblock b through a sim slot + fold bounce."""
                    w = NBW[b]
                    xsq = [
                        xsq_pool.tile([128, 2048], BF16, name="xsq", tag="xsq")[:, 0:w]
                        for k in range(2)
                    ]
                    nc.vector.tensor_mul(xsq[0][:], xt_sb[0][b][:], xt_sb[0][b][:])
                    nc.gpsimd.tensor_mul(xsq[1][:], xt_sb[1][b][:], xt_sb[1][b][:])
                    slot = sim_pool.tile([128, 2048], F32, name="sim", tag="sim")
                    for j in range(w // 512):
                        js = slice(j * 512, (j + 1) * 512)
                        for k in range(2):
                            nc.tensor.matmul(
                                slot[0:1, js],
                                ones_col_bf[:],
                                xsq[k][:, js],
                                start=(k == 0),
                                stop=(k == 1),
                            )
                    nc.vector.tensor_copy(ss_rows[b][:], slot[0:1, 0:w])
                    fold_rsq(
                        ss_rows[b], rsq_rows[b], w, ss_ds[b], rsq_ds[b], f"f{b}"
                    )

                def slot_bc(b):
                    """broadcast rsq row of superblock b + normalize muls."""
                    w = NBW[b]
                    slot = sim_pool.tile([128, 2048], F32, name="sim", tag="sim")
                    for j in range(w // 512):
                        js = slice(j * 512, (j + 1) * 512)
                        nc.tensor.matmul(
                            slot[:, js],
                            ones_row_bf[:],
                            rsq_rows[b][0:1, js],
                            start=True,
                            stop=True,
                        )
                    nc.vector.tensor_copy(bc_sb[b][:], slot[:, 0:w])
                    for k in range(2):
                        eng = nc.vector if k == 0 else nc.gpsimd
                        eng.tensor_mul(zt_sb[k][b][:], xt_sb[k][b][:], bc_sb[b][:])

                def slot_posself():
                    """pos / selfsim partition reductions through one slot."""
                    for k in range(2):
                        nc.vector.tensor_mul(
                            prod_a[k][:], zt_sb[k][0][:, 0:RPC], zt_sb[k][2][:, 0:RPC]
                        )
                        nc.gpsimd.tensor_mul(
                            prod_s[k][:], xt_sb[k][0][:, 0:RPC], zt_sb[k][0][:, 0:RPC]
                        )
                    slot = sim_pool.tile([128, 2048], F32, name="sim", tag="sim")
                    for j in range(2):
                        js = slice(j * 512, (j + 1) * 512)
                        js2 = slice(1024 + j * 512, 1024 + (j + 1) * 512)
                        for k in range(2):
                            nc.tensor.matmul(
                                slot[0:1, js],
                                ones_col[:],
                                prod_a[k][:, js],
                                start=(k == 0),
                                stop=(k == 1),
                            )
                        for k in range(2):
                            nc.tensor.matmul(
                                slot[0:1, js2],
                                ones_col[:],
                                prod_s[k][:, js],
                                start=(k == 0),
                                stop=(k == 1),
                            )
                    nc.vector.tensor_reduce(
                        possum[:], slot[0:1, 0:1024], axis=AX.X, op=ALU.add
                    )
                    # raw selfsim row; the host applies exp(selfs * ra10)
                    # with the exact f32 scale values the GEMM exp used.
                    nc.vector.tensor_copy(selfexp_row[:], slot[0:1, 1024:2048])

                for nb in range(3):
                    w = NBW[nb]
                    if nb < 2:
                        slot_ss(nb + 1)      # bounce runs during this nb's GEMM
                    else:
                        slot_posself()
                    for m in range(M_TILES):
                        ms = slice(m * 128, (m + 1) * 128)
                        st = sim_pool.tile([128, 2048], F32, name="sim", tag="sim")
                        for k in range(2):
                            for j4 in range(w // 512):
                                js = slice(j4 * 512, (j4 + 1) * 512)
                                nc.tensor.matmul(
                                    st[:, js],
                                    xt_sb[k][0][:, ms],
                                    zt_sb[k][nb][:, js],
                                    start=(k == 0),
                                    stop=(k == 1),
                                )
                        e_sb = esb_pool.tile([128, 2048], BF16, name="esb", tag="esb")
                        idx = m * 3 + nb
                        nc.scalar.activation(
                            e_sb[:, 0:w],
                            st[:, 0:w],
                            AF.Exp,
                            scale=ra10_t[:, m : m + 1],
                            accum_out=den_acc[:, idx : idx + 1],
                        )
                        if m == 0:
                            nc.vector.tensor_copy(cacc[nb][:, 0:w], e_sb[:, 0:w])
                        else:
                            nc.vector.tensor_tensor(
                                cacc[nb][:, 0:w], cacc[nb][:, 0:w], e_sb[:, 0:w],
                                op=ALU.add,
                            )
                    if nb < 2:
                        slot_bc(nb + 1)
                    off = sum(NBW[:nb])
                    nc.sync.dma_start(cden_d[:, off : off + w], cacc[nb][:])

            # ---- outputs ----
            nc.sync.dma_start(rden_d[:], den_acc[:])
            nc.sync.dma_start(seout_d[:], selfexp_row[:])
            nc.sync.dma_start(ra_d[:], ra10_t[:])
            nc.sync.dma_start(pos_d[:], possum[:])

    nc.compile()
    return nc


_NC = None


def _get_nc():
    global _NC
    if _NC is None:
        _NC = build_nc()
    return _NC


def make_in_maps(x1, x2):
    import ml_dtypes

    x1 = np.asarray(x1, dtype=np.float32)
    x2 = np.asarray(x2, dtype=np.float32)
    x = np.concatenate([x1, x2], axis=0)              # [8192, 256]
    xT = np.ascontiguousarray(x.T).astype(ml_dtypes.bfloat16)  # [256, 8192]
    in_maps = []
    for c in range(8):
        rot = np.roll(xT, -c * RPC, axis=1)[:, 0:COLS]
        in_maps.append(
            {
                "xt0": np.ascontiguousarray(rot[:128]),
                "xt1": np.ascontiguousarray(rot[128:]),
            }
        )
    return in_maps


def _reduce_host(results):
    """Stand-in for the all-reduce: scatter-add the per-core partials and
    finish the scalar loss."""
    den = np.zeros(TWO_N, dtype=np.float64)
    pos_tot = 0.0
    for c in range(8):
        r = results[c]
        nmax = 3 if c < 4 else 2          # nb=2 (block pair {c, c+4}) owner: c<4
        rden = np.asarray(r["rden"], dtype=np.float64).reshape(128, M_TILES, 3)
        contrib = rden[:, :, 0:nmax].sum(axis=2)        # [p, m]
        den[c * RPC : (c + 1) * RPC] += contrib.T.reshape(RPC)  # row = m*128+p
        colsum = np.asarray(r["cden"], dtype=np.float64).sum(axis=0)  # [5120]
        bmax = 5 if c < 4 else 4
        for b in range(1, bmax):          # b=0 is the diagonal tile: row side only
            g0 = ((c + b) % 8) * RPC
            den[g0 : g0 + RPC] += colsum[b * RPC : (b + 1) * RPC]
        selfs = np.asarray(r["seout"], dtype=np.float64).reshape(RPC)
        ra10 = np.asarray(r["raout"], dtype=np.float64)       # [p, m]
        ra_row = ra10.T.reshape(RPC)                          # row i = m*128+p
        den[c * RPC : (c + 1) * RPC] += 1.0 - np.exp(selfs * ra_row)
        pos_tot += float(np.asarray(r["poso"])[0, 0])
    loss = (np.log(den).sum() - TAU_INV * pos_tot) / TWO_N
    return np.asarray(np.float32(loss))


def _run(x1, x2, trace=False, tmpdir=None):
    from concourse.bass_utils import run_bass_kernel_spmd

    nc = _get_nc()
    in_maps = make_in_maps(x1, x2)
    res = run_bass_kernel_spmd(
        nc, in_maps, list(range(8)), trace=trace, tmpdir=tmpdir
    )
    loss = _reduce_host(res.results)
    return loss, res


def kernel(x1, x2):
    loss, _ = _run(x1, x2)
    return loss
